# revision 9
# baseline (speedup 1.0000x reference)
"""GAT layer kernel for Trainium2, 8 NeuronCores.

Strategy v3 (gather-free, edge-parallel, free-dim reduction):
  - Host: sort nodes by out-degree; tiles = 128 consecutive sorted nodes
    (392 global tiles), dealt round-robin to 8 cores so all cores share one
    column schedule C*_j (max degree within the 8 tiles at position j).
    Within a tile, partition p = node, its edges occupy columns 0..deg-1
    (pads are all-zero x columns).  Host materializes xe = x[dst] per edge
    slot (pure indexing), so the device never gathers.
  - Device, per tile j: Whe/t for every edge slot come from matmuls
    xe_c @ [W | wt] accumulated in PSUM per column; s comes from a tiny
    matmul of the tile's own x against ws.  p = exp(lrelu(s+t) - 4)
    computed as max(exp(e-4), exp(0.2e-4)) (ACT).  Aggregation = DVE
    multiply (p-scaled, pads vanish since xe=0) + tensor_reduce over the
    edge axis; out = num/den with optional deg-0 fallback.
"""

import sys
from dataclasses import dataclass

import numpy as np

sys.path.insert(0, "/opt/trn_rl_repo")

import concourse.bass as bass
import concourse.mybir as mybir
import concourse.tile as tile
from concourse import bacc
from concourse.bass_utils import run_bass_kernel_spmd

N_NODES = 50000
IN_DIM = 256
OUT_DIM = 128
NUM_HEADS = 2

P = 128
NCOL = 2 * OUT_DIM + 2  # matmul out: Wh0|Wh1|t0|t1
SHIFT = 4.0

F32 = mybir.dt.float32
F16 = mybir.dt.float16

PC = 6  # psum columns per chunk (Wh at half-bank granularity, t separate)


@dataclass(frozen=True)
class Cfg:
    n_nodes: int
    n_cores: int
    schedule: tuple  # C*_j per tile position
    fallback: bool
    reps: int = 1

    @property
    def ntiles(self):
        return len(self.schedule)

    @property
    def npc(self):
        return self.ntiles * P  # padded nodes per core

    @property
    def ct(self):
        return sum(self.schedule)


def _apx(ap, dims, off=0):
    """AP keeping ap's partition dim; free dims = (step, count); offset +=
    off elements."""
    return bass.AP(
        ap.tensor, ap.offset + off, [list(ap.ap[0])] + [[s, c] for s, c in dims]
    )


def host_prep(x, edge_index, W_w, W_b, a, n_cores=8):
    x = np.asarray(x, dtype=np.float32)
    edge_index = np.asarray(edge_index)
    W_w = np.asarray(W_w, dtype=np.float32)
    W_b = np.asarray(W_b, dtype=np.float32)
    a = np.asarray(a, dtype=np.float32)
    assert np.abs(W_b).max() == 0.0, "nonzero bias not supported"

    n_nodes, in_dim = x.shape
    D = OUT_DIM
    src = np.asarray(edge_index[0], dtype=np.int64)
    dst = np.asarray(edge_index[1], dtype=np.int64)

    # folded params
    a_src, a_dst = a[:D], a[D:]
    ws = np.stack([W_w[:, 0:D] @ a_src, W_w[:, D : 2 * D] @ a_src], axis=1)  # [in,2]
    wt = np.stack([W_w[:, 0:D] @ a_dst, W_w[:, D : 2 * D] @ a_dst], axis=1)
    wedge = np.concatenate([W_w, wt], axis=1).astype(np.float16)  # [in, 258]
    wown = ws.astype(np.float16)  # [in, 2]
    # pad-column vector: v @ wt == (-80, -80) so pad slots get p16 == 0
    G = wt.T @ wt
    ab = np.linalg.solve(G, np.array([-80.0, -80.0]))
    vpad = (wt @ ab).astype(np.float32)  # [in]

    deg = np.bincount(src, minlength=n_nodes)
    fallback = bool(deg.min() == 0)

    ntiles = (n_nodes + n_cores * P - 1) // (n_cores * P)  # 49
    ntile_tot = n_cores * ntiles
    npad = ntile_tot * P

    order_n = np.argsort(-deg, kind="stable")  # sorted node ids, deg desc
    deg_pad = np.zeros(npad, dtype=np.int64)
    deg_pad[:n_nodes] = deg[order_n]
    tile_max = deg_pad.reshape(ntile_tot, P).max(axis=1)
    sched = tile_max.reshape(ntiles, n_cores).max(axis=1)
    sched = np.maximum(sched, 1).astype(np.int64)
    cfg = Cfg(
        n_nodes=n_nodes,
        n_cores=n_cores,
        schedule=tuple(int(c) for c in sched),
        fallback=fallback,
    )
    off = np.zeros(ntiles, dtype=np.int64)
    off[1:] = np.cumsum(sched)[:-1]
    CT = int(sched.sum())
    S = CT * P

    # node -> (core, j, p):  sorted position q -> global tile q//128,
    # partition q%128; global tile T -> core T % n_cores, position T//n_cores
    node_q = np.empty(n_nodes, dtype=np.int64)
    node_q[order_n] = np.arange(n_nodes)
    node_tile = node_q // P
    node_p = node_q % P
    node_core = node_tile % n_cores
    node_j = node_tile // n_cores

    # edge ranks within node (edges sorted by src)
    eorder = np.argsort(src, kind="stable")
    src_s = src[eorder]
    dst_s = dst[eorder]
    starts = np.zeros(n_nodes, dtype=np.int64)
    starts[1:] = np.cumsum(deg)[:-1]
    rank = np.arange(len(src_s)) - starts[src_s]

    e_core = node_core[src_s]
    e_slot = (off[node_j[src_s]] + rank) * P + node_p[src_s]

    xT16 = np.ascontiguousarray(x.T).astype(np.float16)  # [in, n_nodes]

    shared = {"wedge": wedge, "wown": wown}
    per_core = []
    for k in range(n_cores):
        m = e_core == k
        slots = e_slot[m]
        dcol = dst_s[m]
        xe = np.broadcast_to(
            vpad.astype(np.float16)[:, None], (in_dim, S)
        ).copy()
        xe[:, slots] = xT16[:, dcol]
        mine = node_core == k
        nodes_k = np.nonzero(mine)[0]
        rows_k = node_j[nodes_k] * P + node_p[nodes_k]
        xo = np.zeros((in_dim, ntiles * P), dtype=np.float16)
        xo[:, rows_k] = xT16[:, nodes_k]
        per_core.append(
            {"xe": xe, "xo": xo, "_nodes": nodes_k, "_rows": rows_k}
        )
    return cfg, shared, per_core


def build_program(cfg: Cfg):
    ntiles = cfg.ntiles
    sched = cfg.schedule
    Cmax = max(sched)
    CT = cfg.ct
    S = CT * P
    D = OUT_DIM
    nc = bacc.Bacc("TRN2", target_bir_lowering=False, debug=False)

    xe_d = nc.dram_tensor("xe", [IN_DIM, S], F16, kind="ExternalInput")
    xo_d = nc.dram_tensor("xo", [IN_DIM, ntiles * P], F16, kind="ExternalInput")
    wedge_d = nc.dram_tensor("wedge", [IN_DIM, NCOL], F16, kind="ExternalInput")
    wown_d = nc.dram_tensor("wown", [IN_DIM, 2], F16, kind="ExternalInput")
    out_d = nc.dram_tensor("out", [ntiles * P, 2 * D], F32, kind="ExternalOutput")

    OGRP = 8

    with tile.TileContext(nc) as tc:
        with (
            tc.tile_pool(name="const", bufs=1) as constp,
            tc.tile_pool(name="xej", bufs=2) as xep,
            tc.tile_pool(name="xoj", bufs=2) as xop,
            tc.tile_pool(name="pse", bufs=2, space="PSUM") as psep,
            tc.tile_pool(name="rhs", bufs=2) as rhsp,
            tc.tile_pool(name="sml", bufs=4) as smlp,
            tc.tile_pool(name="p16", bufs=2) as p16p,
            tc.tile_pool(name="agg", bufs=2) as aggp,
            tc.tile_pool(name="og", bufs=2) as ogp,
        ):
            wedge_t = constp.tile([P, 2, NCOL], F16, tag="wedge")
            nc.sync.dma_start(
                out=wedge_t[:],
                in_=wedge_d[:, :].rearrange("(kt p) c -> p kt c", p=P),
            )
            wown_t = constp.tile([P, 2, 2], F16, tag="wown")
            nc.sync.dma_start(
                out=wown_t[:], in_=wown_d[:, :].rearrange("(kt p) c -> p kt c", p=P)
            )
            shift_t = constp.tile([P, 1], F32, tag="shift")
            nc.vector.memset(shift_t[:], -SHIFT)

            og = None
            jlist = [jj for _ in range(cfg.reps) for jj in range(ntiles)]
            for idx, j in enumerate(jlist):
                C = sched[j]
                off = sum(sched[:j])
                g = idx % OGRP
                if g == 0:
                    og = ogp.tile([P, OGRP, 2 * D], F32, tag="og")

                xej = xep.tile([P, 2, Cmax * P], F16, tag="xej")
                for kt in range(2):
                    nc.sync.dma_start(
                        out=xej[:, kt, 0 : C * P],
                        in_=xe_d[kt * P : (kt + 1) * P, off * P : (off + C) * P],
                    )
                xoj = xop.tile([P, 2, P], F16, tag="xoj")
                for kt in range(2):
                    nc.sync.dma_start(
                        out=xoj[:, kt, :],
                        in_=xo_d[kt * P : (kt + 1) * P, j * P : (j + 1) * P],
                    )

                s_sb = smlp.tile([P, 2], F32, tag="s_sb")
                p16j = p16p.tile([P, Cmax, 2], F16, tag="p16j")
                # rhs2 layout: [h, d, c] with c innermost (stride 1)
                rhs2 = rhsp.tile([P, 2, D, Cmax], F16, tag="rhs2")

                c0 = 0
                while c0 < C:
                    w = min(PC, C - c0)
                    pse = psep.tile([P, PC, 2 * D], F32, tag="pse")
                    pst = psep.tile([P, PC + 1, 2], F32, tag="pst")
                    if c0 == 0:
                        # s_own into pst slot PC, then to SBUF
                        for kt in range(2):
                            nc.tensor.matmul(
                                out=pst[:, PC, :],
                                lhsT=xoj[:, kt, :],
                                rhs=wown_t[:, kt, :],
                                start=(kt == 0),
                                stop=(kt == 1),
                            )
                        nc.vector.tensor_copy(out=s_sb[:], in_=pst[:, PC, :])
                    for ci in range(w):
                        for kt in range(2):
                            nc.tensor.matmul(
                                out=pse[:, ci, :],
                                lhsT=xej[:, kt, (c0 + ci) * P : (c0 + ci + 1) * P],
                                rhs=wedge_t[:, kt, 0 : 2 * D],
                                start=(kt == 0),
                                stop=(kt == 1),
                            )
                            nc.tensor.matmul(
                                out=pst[:, ci, :],
                                lhsT=xej[:, kt, (c0 + ci) * P : (c0 + ci + 1) * P],
                                rhs=wedge_t[:, kt, 2 * D : NCOL],
                                start=(kt == 0),
                                stop=(kt == 1),
                            )
                    # e = t + s  [128, w, 2]
                    e2 = smlp.tile([P, PC, 2], F32, tag="e2")
                    nc.vector.tensor_tensor(
                        out=e2[:, 0:w, :],
                        in0=_apx(pst[:], [(2, w), (1, 2)]),
                        in1=_apx(s_sb[:], [(0, w), (1, 2)]),
                        op=mybir.AluOpType.add,
                    )
                    # p = max(exp(e-4), exp(0.2e-4))
                    pa = smlp.tile([P, PC, 2], F16, tag="pa")
                    nc.scalar.activation(
                        out=pa[:, 0:w, :].rearrange("p c h -> p (c h)"),
                        in_=e2[:, 0:w, :].rearrange("p c h -> p (c h)"),
                        func=mybir.ActivationFunctionType.Exp,
                        bias=shift_t[:, 0:1],
                        scale=1.0,
                    )
                    pb = smlp.tile([P, PC, 2], F16, tag="pb")
                    nc.scalar.activation(
                        out=pb[:, 0:w, :].rearrange("p c h -> p (c h)"),
                        in_=e2[:, 0:w, :].rearrange("p c h -> p (c h)"),
                        func=mybir.ActivationFunctionType.Exp,
                        bias=shift_t[:, 0:1],
                        scale=0.2,
                    )
                    nc.vector.tensor_tensor(
                        out=p16j[:, c0 : c0 + w, :],
                        in0=pa[:, 0:w, :],
                        in1=pb[:, 0:w, :],
                        op=mybir.AluOpType.max,
                    )
                    # rhs2[:, h, d, c0+ci] = pse[:, ci, h*128+d] * p16[c0+ci, h]
                    nc.vector.tensor_tensor(
                        out=_apx(rhs2[:], [(D * Cmax, 2), (Cmax, D), (1, w)], off=c0),
                        in0=_apx(pse[:], [(D, 2), (1, D), (2 * D, w)]),
                        in1=_apx(p16j[:], [(1, 2), (0, D), (2, w)], off=c0 * 2),
                        op=mybir.AluOpType.mult,
                    )
                    c0 += w

                # aggregate: agg[(h d)] = sum_c rhs2[h, d, c]  (f16, c packed)
                agg = aggp.tile([P, 2 * D], F16, tag="agg")
                with nc.allow_low_precision(reason="f16 sums within 2e-2 tol"):
                    nc.vector.tensor_reduce(
                        out=agg[:],
                        in_=_apx(rhs2[:], [(Cmax, 2 * D), (1, C)]),
                        axis=mybir.AxisListType.X,
                        op=mybir.AluOpType.add,
                    )
                den = smlp.tile([P, 2], F32, tag="den")
                nc.vector.tensor_reduce(
                    out=den[:],
                    in_=_apx(p16j[:], [(1, 2), (2, C)]),
                    axis=mybir.AxisListType.X,
                    op=mybir.AluOpType.add,
                )
                dns = smlp.tile([P, 2], F32, tag="dns")
                nc.vector.tensor_scalar(
                    out=dns[:], in0=den[:], scalar1=1e-30, scalar2=None,
                    op0=mybir.AluOpType.max,
                )
                rcp = smlp.tile([P, 2], F32, tag="rcp")
                nc.vector.reciprocal(out=rcp[:], in_=dns[:])
                for h in range(2):
                    nc.scalar.activation(
                        out=og[:, g, h * D : (h + 1) * D],
                        in_=agg[:, h * D : (h + 1) * D],
                        func=mybir.ActivationFunctionType.Copy,
                        scale=rcp[:, h : h + 1],
                    )
                if cfg.fallback:
                    psw = pssp.tile([P, NCOL], F32, tag="psw")
                    for kt in range(2):
                        nc.tensor.matmul(
                            out=psw[:],
                            lhsT=xoj[:, kt, :],
                            rhs=wedge_t[:, kt, :],
                            start=(kt == 0),
                            stop=(kt == 1),
                        )
                    nm = smlp.tile([P, 1], F32, tag="nm")
                    nc.vector.tensor_scalar(
                        out=nm[:], in0=den[:, 0:1], scalar1=0.0, scalar2=None,
                        op0=mybir.AluOpType.is_le,
                    )
                    fbm = smlp.tile([P, 2 * D], F32, tag="fbm")
                    nc.vector.tensor_scalar(
                        out=fbm[:], in0=psw[:, 0 : 2 * D],
                        scalar1=nm[:, 0:1], scalar2=None,
                        op0=mybir.AluOpType.mult,
                    )
                    nc.vector.tensor_tensor(
                        out=og[:, g, :], in0=og[:, g, :], in1=fbm[:],
                        op=mybir.AluOpType.add,
                    )

                if g == OGRP - 1 or idx == len(jlist) - 1:
                    j0 = (idx - g) % ntiles
                    nc.sync.dma_start(
                        out=out_d[j0 * P : (j0 + g + 1) * P, :].rearrange(
                            "(g p) c -> p g c", p=P
                        ),
                        in_=og[:, 0 : g + 1, :],
                    )

    nc.compile()
    return nc


_prog_cache = {}


def kernel(x, edge_index, W_w, W_b, a):
    cfg, shared, per_core = host_prep(x, edge_index, W_w, W_b, a, n_cores=8)
    if cfg not in _prog_cache:
        _prog_cache[cfg] = build_program(cfg)
    nc = _prog_cache[cfg]
    in_maps = [
        {kk: v for kk, v in {**shared, **pc}.items() if not kk.startswith("_")}
        for pc in per_core
    ]
    res = run_bass_kernel_spmd(nc, in_maps, list(range(cfg.n_cores)))
    out = np.zeros((cfg.n_nodes, 2 * OUT_DIM), dtype=np.float32)
    for k in range(cfg.n_cores):
        pc = per_core[k]
        out[pc["_nodes"]] = res.results[k]["out"][pc["_rows"]]
    return out
